# revision 1
# baseline (speedup 1.0000x reference)
"""Trainium2 Bass kernel for nn_CrossAttention_36309653521078.

Math notes:
  - seq_len == 1 => softmax over a single key is identically 1.0, so
    attn == V exactly. Q/K/score computation is dead code (bitwise
    identical output without it).
  - The chain per row b:
        V    = h_s @ Wv_flat + bv_flat          [B, 512]
        x1   = V @ Wo + bo + h_g
        ha   = LN(x1) * g1 + b1_ln
        mlp  = gelu(ha @ W1 + b1) @ W2 + b2
        out  = LN(mlp + ha) * g2 + b2_ln
  - Device works feature-major (activations stored transposed [D, B]):
    every matmul takes W[din, dout] as the stationary operand and the
    activation [din, b] as the moving operand. The host pre-transposes
    h_s / h_g once and transposes the output back.
  - Matmuls run in float32r (fp32 with an 11-bit-mantissa datapath, ~4x
    the fp32 matmul rate). All tensors feeding a matmul are declared
    float32r; engines round on write, DMA moves raw fp32 bits.
Sharding: pure data parallelism over the batch dim across 8 cores.
"""

import numpy as np

import concourse.bass as bass
import concourse.mybir as mybir
import concourse.tile as tile
from concourse.bass_utils import run_bass_kernel_spmd

F32 = mybir.dt.float32

N_CORES = 8
B = 16384
G_DIM = 512
S_DIM = 3072
HID = 512
H2 = 1024
BL = B // N_CORES      # rows per core
NB = 512               # batch-tile (moving free dim; fp32 max 512)
NBT = BL // NB         # batch tiles per core
KSUB = 4               # h_s k-subtiles (of 128) per streamed DMA chunk
EPS = 1e-5

SK = S_DIM // 128      # 24
KO = HID // 128        # 4
MH = H2 // 128         # 8

# matmul dtype: float32r streams at bf16 rate (1 cyc/row for N>=256) with
# an 11-bit mantissa; plain float32 is 4x slower.
MM_DT = mybir.dt.float32r

# consts tile column layout (each entry is [128, n] chunks of a vector)
_C_BV = 0              # bv_flat          [512]  -> cols 0:4
_C_B1 = 4              # b1               [1024] -> cols 4:12
_C_B2 = 12             # b2               [512]  -> cols 12:16
_C_L1G = 16            # ln1_g            cols 16:20
_C_L1B = 20            # ln1_b            cols 20:24
_C_L2G = 24            # ln2_g            cols 24:28
_C_L2B = 28            # ln2_b            cols 28:32
_C_N = 32


def _split_multi_waits(nc):
    """The walrus build here rejects >1 sync-wait on several instruction
    codegen structs (Drain/CTRL, fused-LDW matmul). Hoist extra waits onto
    single-wait NOPs inserted just before the owning instruction."""
    for blk in nc.m.functions[0].blocks:
        insts = list(blk.instructions)
        out, changed, k = [], False, 0
        for inst in insts:
            si = inst.sync_info
            waits = list(si.on_wait) if si and si.on_wait else []
            if len(waits) > 1:
                for w in waits[:-1]:
                    out.append(mybir.InstNoOp(
                        name=f"wsplit-{blk.name}-{k}",
                        engine=inst.engine,
                        bass_nofuse=True,
                        sync_info=mybir.SyncInfo(on_wait=[w], on_update=[]),
                    ))
                    k += 1
                si.on_wait = [waits[-1]]
                changed = True
            out.append(inst)
        if changed:
            blk.instructions = out


def _f32view(ap):
    """fp32 view of a (possibly float32r) AP for elementwise reads."""
    return ap.bitcast(F32) if ap.dtype == mybir.dt.float32r else ap


def _layernorm_feature_major(nc, pools, x, consts, gcol, bcol, nchunks):
    """In-place LN over the partition (feature) axis of x [128, nchunks, NB].

    Stats use an all-ones [128,128] stationary matmul: every output
    partition row receives the column sum, i.e. the partition reduction
    arrives already broadcast across partitions. (f32r matmuls require a
    full-partition destination anyway.)
    """
    psum, stat_pool, xsq_pool, ones128, eps_col = pools
    inv_n = 1.0 / (nchunks * 128)

    sumB = psum.tile([128, NB], F32, tag="psum_mm", name="sumB")
    for j in range(nchunks):
        nc.tensor.matmul(sumB, ones128, x[:, j, :],
                         start=(j == 0), stop=(j == nchunks - 1))
    sqB = psum.tile([128, NB], F32, tag="psum_mm", name="sqB")
    for j in range(nchunks):
        xsq = xsq_pool.tile([128, NB], MM_DT, tag="xsq", name=f"xsq{j}")
        nc.vector.tensor_mul(xsq, _f32view(x[:, j, :]), _f32view(x[:, j, :]))
        nc.tensor.matmul(sqB, ones128, xsq,
                         start=(j == 0), stop=(j == nchunks - 1))

    # muB = mean, rB = 1/sqrt(var+eps), all [128, NB] (broadcast rows)
    muB = stat_pool.tile([128, NB], F32, tag="muB", name="muB")
    nc.scalar.activation(muB, sumB,
                         mybir.ActivationFunctionType.Copy, scale=inv_n)
    rB = stat_pool.tile([128, NB], F32, tag="rB", name="rB")
    nc.scalar.activation(rB, sqB,
                         mybir.ActivationFunctionType.Copy, scale=inv_n)
    musqB = stat_pool.tile([128, NB], F32, tag="musqB", name="musqB")
    nc.vector.tensor_mul(musqB, muB, muB)
    nc.vector.tensor_sub(rB, rB, musqB)
    nc.scalar.activation(rB, rB,
                         mybir.ActivationFunctionType.Sqrt, bias=eps_col)
    nc.vector.reciprocal(rB, rB)

    # x = ((x - muB) * rB) * g + beta  (g, beta per-feature = per-partition)
    for j in range(nchunks):
        nc.vector.tensor_sub(x[:, j, :], _f32view(x[:, j, :]), muB)
        nc.vector.tensor_mul(x[:, j, :], _f32view(x[:, j, :]), rB)
        nc.scalar.activation(
            x[:, j, :], _f32view(x[:, j, :]),
            mybir.ActivationFunctionType.Identity,
            bias=consts[:, bcol + j: bcol + j + 1],
            scale=consts[:, gcol + j: gcol + j + 1],
        )


def build_nc(reps: int = 1, split_waits: bool = True, timing: bool = False):
    """reps>1 repeats the whole per-core body (for differential timing).
    timing=True shrinks the DRAM activations (one batch-tile, re-read for
    every batch-tile) and adds a reps-sized marker output so that timing
    variants can't collide in any executable cache. split_waits must be
    True for HW (walrus); CoreSim needs False."""
    nc = bass.Bass("TRN2", target_bir_lowering=False, debug=False)

    bl = NB if timing else BL
    hs_rows = KSUB * 128 if timing else S_DIM
    wv_rows = S_DIM // 8 if timing else S_DIM
    hsT = nc.dram_tensor("hsT", [hs_rows, bl], MM_DT, kind="ExternalInput").ap()
    hgT = nc.dram_tensor("hgT", [HID, bl], F32, kind="ExternalInput").ap()
    wv = nc.dram_tensor("wv", [wv_rows, HID], MM_DT, kind="ExternalInput").ap()
    wo = nc.dram_tensor("wo", [HID, HID], MM_DT, kind="ExternalInput").ap()
    w1 = nc.dram_tensor("w1", [HID, H2], MM_DT, kind="ExternalInput").ap()
    w2 = nc.dram_tensor("w2", [H2, HID], MM_DT, kind="ExternalInput").ap()
    cst = nc.dram_tensor("cst", [128, _C_N], F32, kind="ExternalInput").ap()
    outT = nc.dram_tensor("outT", [HID, bl], MM_DT, kind="ExternalOutput").ap()
    mark = None
    if timing:
        mark = nc.dram_tensor("mark", [1, 8 * reps], F32,
                              kind="ExternalOutput").ap()

    n_kg = hs_rows // (KSUB * 128)
    hsT_t = hsT.rearrange("(kg kk p) b -> kg p kk b", kk=KSUB, p=128)
    hgT_t = hgT.rearrange("(c p) b -> p c b", p=128)
    outT_t = outT.rearrange("(c p) b -> p c b", p=128)

    with tile.TileContext(nc) as tc:
        with (
            nc.allow_low_precision(
                reason="float32r matmul inputs: 11-bit mantissa by design"),
            tc.tile_pool(name="weights", bufs=1) as wpool,
            tc.tile_pool(name="hs", bufs=3) as hs_pool,
            tc.tile_pool(name="hg", bufs=2) as hg_pool,
            tc.tile_pool(name="v", bufs=6) as v_pool,
            tc.tile_pool(name="act", bufs=2) as act_pool,
            tc.tile_pool(name="g", bufs=6) as g_pool,
            tc.tile_pool(name="xsq", bufs=3) as xsq_pool,
            tc.tile_pool(name="stat", bufs=2) as stat_pool,
            tc.tile_pool(name="out", bufs=2) as out_pool,
            tc.tile_pool(name="psum", bufs=8, space="PSUM") as psum,
        ):
            # ---- resident weights / constants ----
            wv_sb = wpool.tile([128, SK, HID], MM_DT)
            wv_r = wv.rearrange("(kc p) n -> p kc n", p=128)
            n_wv_kc = wv_rows // 128
            for j0 in range(0, SK, n_wv_kc):
                nc.sync.dma_start(out=wv_sb[:, j0:j0 + n_wv_kc, :], in_=wv_r)
            wo_sb = wpool.tile([128, KO, HID], MM_DT)
            nc.sync.dma_start(out=wo_sb, in_=wo.rearrange("(kc p) n -> p kc n", p=128))
            w1_sb = wpool.tile([128, KO, H2], MM_DT)
            nc.sync.dma_start(out=w1_sb, in_=w1.rearrange("(kc p) n -> p kc n", p=128))
            w2_sb = wpool.tile([128, MH, HID], MM_DT)
            nc.sync.dma_start(out=w2_sb, in_=w2.rearrange("(kc p) n -> p kc n", p=128))
            consts = wpool.tile([128, _C_N], F32)
            nc.sync.dma_start(out=consts, in_=cst)
            # memset can't write float32r; produce ones via an ACT copy
            ones_f = wpool.tile([128, 128], F32)
            nc.vector.memset(ones_f, 1.0)
            ones128 = wpool.tile([128, 128], MM_DT)
            nc.scalar.activation(ones128, ones_f,
                                 mybir.ActivationFunctionType.Copy)
            eps_col = wpool.tile([128, 1], F32)
            nc.vector.memset(eps_col, EPS)
            mark_sb = None
            if timing:
                mark_sb = wpool.tile([1, 8], F32)
                nc.vector.memset(mark_sb, 1.0)

            ln_pools = (psum, stat_pool, xsq_pool, ones128, eps_col)

            for rep in range(reps):
              for bt in range(NBT):
                bsl = slice(0, NB) if timing else slice(bt * NB, (bt + 1) * NB)

                # ---- V = h_s @ Wv + bv  (feature-major V^T in sbuf) ----
                psum_v = [psum.tile([128, NB], F32, tag="psum_mm",
                                    name=f"psv{rep}_{bt}_{i}") for i in range(KO)]
                for kg in range(SK // KSUB):
                    hs_t = hs_pool.tile([128, KSUB, NB], MM_DT, name="hs_t")
                    nc.sync.dma_start(out=hs_t, in_=hsT_t[kg % n_kg, :, :, bsl])
                    for kk in range(KSUB):
                        k = kg * KSUB + kk
                        for m in range(KO):
                            nc.tensor.matmul(
                                psum_v[m],
                                wv_sb[:, k, m * 128:(m + 1) * 128],
                                hs_t[:, kk, :],
                                start=(k == 0), stop=(k == SK - 1),
                            )
                v_sb = []
                for m in range(KO):
                    v = v_pool.tile([128, NB], MM_DT, tag="v",
                                    name=f"v{rep}_{bt}_{m}")
                    nc.scalar.activation(v, psum_v[m],
                                         mybir.ActivationFunctionType.Identity,
                                         bias=consts[:, _C_BV + m: _C_BV + m + 1])
                    v_sb.append(v)

                # ---- x1 = V @ Wo (+ bo + h_g, bo folded into hgT host-side) ----
                hg_t = hg_pool.tile([128, KO, NB], F32, name="hg_t")
                nc.sync.dma_start(out=hg_t, in_=hgT_t[:, :, bsl])
                x1 = act_pool.tile([128, KO, NB], MM_DT, tag="x1", name="x1")
                for m in range(KO):
                    po = psum.tile([128, NB], F32, tag="psum_mm",
                                   name=f"pso{rep}_{bt}_{m}")
                    for k in range(KO):
                        nc.tensor.matmul(
                            po,
                            wo_sb[:, k, m * 128:(m + 1) * 128],
                            v_sb[k],
                            start=(k == 0), stop=(k == KO - 1),
                        )
                    nc.vector.tensor_add(x1[:, m, :], po, hg_t[:, m, :])

                # ---- LN1 -> h_attn (in place on x1) ----
                _layernorm_feature_major(nc, ln_pools, x1, consts,
                                         _C_L1G, _C_L1B, KO)

                # ---- g = gelu(h_attn @ W1 + b1) ----
                g_sb = []
                for m in range(MH):
                    p1 = psum.tile([128, NB], F32, tag="psum_mm",
                                   name=f"ps1{rep}_{bt}_{m}")
                    for k in range(KO):
                        nc.tensor.matmul(
                            p1,
                            w1_sb[:, k, m * 128:(m + 1) * 128],
                            x1[:, k, :],
                            start=(k == 0), stop=(k == KO - 1),
                        )
                    g = g_pool.tile([128, NB], MM_DT, tag="g",
                                    name=f"g{rep}_{bt}_{m}")
                    nc.scalar.activation(g, p1,
                                         mybir.ActivationFunctionType.Gelu,
                                         bias=consts[:, _C_B1 + m: _C_B1 + m + 1])
                    g_sb.append(g)

                # ---- x2 = g @ W2 + b2 + h_attn ----
                psum_2 = [psum.tile([128, NB], F32, tag="psum_mm",
                                    name=f"ps2{rep}_{bt}_{i}") for i in range(KO)]
                for k in range(MH):
                    for m in range(KO):
                        nc.tensor.matmul(
                            psum_2[m],
                            w2_sb[:, k, m * 128:(m + 1) * 128],
                            g_sb[k],
                            start=(k == 0), stop=(k == MH - 1),
                        )
                x2 = out_pool.tile([128, KO, NB], MM_DT, tag="x2", name="x2")
                for m in range(KO):
                    nc.scalar.activation(x2[:, m, :], psum_2[m],
                                         mybir.ActivationFunctionType.Identity,
                                         bias=consts[:, _C_B2 + m: _C_B2 + m + 1])
                    nc.vector.tensor_add(x2[:, m, :], _f32view(x2[:, m, :]),
                                         _f32view(x1[:, m, :]))

                # ---- LN2 -> out (in place on x2) ----
                _layernorm_feature_major(nc, ln_pools, x2, consts,
                                         _C_L2G, _C_L2B, KO)

                nc.sync.dma_start(out=outT_t[:, :, bsl], in_=x2)

              if timing:
                nc.sync.dma_start(out=mark[0:1, 8 * rep: 8 * (rep + 1)],
                                  in_=mark_sb)

    if split_waits:
        _split_multi_waits(nc)
    return nc


def _chunk_cols(vec):
    """[n*128] -> [128, n] with column j = vec[j*128:(j+1)*128]."""
    return np.ascontiguousarray(vec.reshape(-1, 128).T.astype(np.float32))


def _make_consts(inputs):
    b1 = np.asarray(inputs["b1"], np.float32)
    b2 = np.asarray(inputs["b2"], np.float32)
    bv_flat = np.asarray(inputs["bv"], np.float32).reshape(HID)
    cst = np.concatenate(
        [
            _chunk_cols(bv_flat),
            _chunk_cols(b1),
            _chunk_cols(b2),
            _chunk_cols(np.asarray(inputs["ln1_g"], np.float32)),
            _chunk_cols(np.asarray(inputs["ln1_b"], np.float32)),
            _chunk_cols(np.asarray(inputs["ln2_g"], np.float32)),
            _chunk_cols(np.asarray(inputs["ln2_b"], np.float32)),
        ],
        axis=1,
    )
    assert cst.shape == (128, _C_N)
    return cst


def _shared_weights(inputs):
    Wv = np.asarray(inputs["Wv"], np.float32)
    return {
        "wv": np.ascontiguousarray(Wv.transpose(1, 0, 2).reshape(S_DIM, HID)),
        "wo": np.ascontiguousarray(np.asarray(inputs["Wo"], np.float32)),
        "w1": np.ascontiguousarray(np.asarray(inputs["W1"], np.float32)),
        "w2": np.ascontiguousarray(np.asarray(inputs["W2"], np.float32)),
        "cst": _make_consts(inputs),
    }


def _prepare_in_maps(inputs):
    h_g = np.asarray(inputs["h_g"], np.float32)
    h_s = np.asarray(inputs["h_s"], np.float32)
    bo = np.asarray(inputs["bo"], np.float32)
    shared = _shared_weights(inputs)
    in_maps = []
    for c in range(N_CORES):
        rows = slice(c * BL, (c + 1) * BL)
        in_maps.append({
            "hsT": np.ascontiguousarray(h_s[rows].T),
            # fold bo into the h_g residual: x1 = V@Wo + (h_g + bo)
            "hgT": np.ascontiguousarray(h_g[rows].T + bo[:, None]),
            **shared,
        })
    return in_maps


def _prepare_timing_in_maps(inputs):
    h_g = np.asarray(inputs["h_g"], np.float32)
    h_s = np.asarray(inputs["h_s"], np.float32)
    bo = np.asarray(inputs["bo"], np.float32)
    shared = _shared_weights(inputs)
    m = {
        "hsT": np.ascontiguousarray(h_s[:NB, :KSUB * 128].T),
        "hgT": np.ascontiguousarray(h_g[:NB].T + bo[:, None]),
        **shared,
    }
    m["wv"] = np.ascontiguousarray(m.pop("wv")[: S_DIM // 8])
    return [dict(m) for _ in range(N_CORES)]


def _assemble(results):
    return np.ascontiguousarray(
        np.concatenate([r["outT"].T for r in results], axis=0))


def run(inputs, trace=False):
    nc = build_nc()
    in_maps = _prepare_in_maps(inputs)
    res = run_bass_kernel_spmd(nc, in_maps, list(range(N_CORES)), trace=trace)
    return _assemble(res.results), res


def kernel(**inputs):
    out, _ = run(inputs, trace=False)
    return out



# revision 51
# speedup vs baseline: 1.0918x; 1.0918x over previous
"""Trainium2 Bass kernel for nn_CrossAttention_36309653521078.

Math notes:
  - seq_len == 1 => softmax over a single key is identically 1.0, so
    attn == V exactly. Q/K/score computation is dead code.
  - Per row b (feature-major on device, batch in the moving free dim):
        V    = h_s @ Wv_flat + bv          [B, 512]
        x1   = V @ Wo + (h_g + bo)
        xhat = (x1 - mu1) * rstd1                     (plain normalize)
        W1-path: gelu(xhat @ W1' + b1')  with W1' = g1*W1,
                 b1' = b1 + ln1_b @ W1             (LN1 affine folded in)
        x1r  = xhat * g1 + (ln1_b + b2)               (residual carry,
                                                       b2 folded in)
        x2   = gelu(...) @ W2 + x1r
        out  = (x2 - mu2) * rstd2 * g2 + ln2_b
  - Everything bf16 except PSUM accumulation and LN statistics (fp32).
  - LN stats use a (1/512)-scaled all-ones stationary matmul: the psum
    rows receive the feature-mean directly, broadcast across partitions.
  - Software pipelining: tile t's tail (x2 add, x^2, LN2 stats,
    normalize, store) is emitted after tile t+1's V matmuls, so the
    PE never waits for the ACT/DVE tail chain at tile boundaries.
  - DMA queues: h_s/h_g stream on the SP HWDGE ring; weights and output
    stores ride the ACT ring so they never head-of-line-block inputs.
Sharding: pure data parallelism over the batch dim across 8 cores.
"""

import numpy as np

import concourse.bass as bass
import concourse.mybir as mybir
import concourse.tile as tile
from concourse.bass_utils import run_bass_kernel_spmd

F32 = mybir.dt.float32
AF = mybir.ActivationFunctionType

N_CORES = 8
B = 16384
G_DIM = 512
S_DIM = 3072
HID = 512
H2 = 1024
BL = B // N_CORES      # rows per core
NB = 512               # batch-tile (moving free dim)
NBT = BL // NB         # batch tiles per core
KSUB = 4               # h_s k-subtiles (of 128) per streamed DMA chunk
EPS = 1e-5

SK = S_DIM // 128      # 24
KO = HID // 128        # 4
MH = H2 // 128         # 8

MM_DT = mybir.dt.bfloat16

# consts tile column layout (each entry is [128, n] chunks of a vector)
_C_BV = 0              # bv_flat              [512]  -> cols 0:4
_C_B1 = 4              # b1' = b1 + ln1_b@W1  [1024] -> cols 4:12
_C_X1G = 12            # g1  (x1r affine scale)      -> cols 12:16
_C_X1B = 16            # ln1_b + b2 (x1r affine bias)-> cols 16:20
_C_L2G = 20            # ln2_g                       -> cols 20:24
_C_L2B = 24            # ln2_b                       -> cols 24:28
_C_W1S = 28            # -colsum(W1') [1024]         -> cols 28:36
_C_N = 36


def _split_multi_waits(nc):
    """The walrus build here rejects >1 sync-wait on several instruction
    codegen structs (Drain/CTRL, fused-LDW matmul). Hoist extra waits onto
    single-wait NOPs inserted just before the owning instruction."""
    for blk in nc.m.functions[0].blocks:
        insts = list(blk.instructions)
        out, changed, k = [], False, 0
        for inst in insts:
            si = inst.sync_info
            waits = list(si.on_wait) if si and si.on_wait else []
            if len(waits) > 1:
                for w in waits[:-1]:
                    out.append(mybir.InstNoOp(
                        name=f"wsplit-{blk.name}-{k}",
                        engine=inst.engine,
                        bass_nofuse=True,
                        sync_info=mybir.SyncInfo(on_wait=[w], on_update=[]),
                    ))
                    k += 1
                si.on_wait = [waits[-1]]
                changed = True
            out.append(inst)
        if changed:
            blk.instructions = out


def build_nc(reps: int = 1, split_waits: bool = True, timing: bool = False,
             mark_reps: bool = False):
    """reps>1 repeats the whole per-core body (for differential timing).
    timing=True shrinks the DRAM activations; a reps-sized marker output
    keeps timing variants from colliding in executable caches (cache keys
    ignore the program body when tensor shapes match). mark_reps=True adds
    the marker for full-size builds too. split_waits must be True for HW
    (walrus); CoreSim needs False."""
    nc = bass.Bass("TRN2", target_bir_lowering=False, debug=False)
    mark_out = timing or mark_reps

    bl = NB if timing else BL
    hs_rows = KSUB * 128 if timing else S_DIM
    wv_rows = S_DIM // 8 if timing else S_DIM
    hsT = nc.dram_tensor("hsT", [hs_rows, bl], MM_DT, kind="ExternalInput").ap()
    hgT = nc.dram_tensor("hgT", [HID, bl], MM_DT, kind="ExternalInput").ap()
    wv = nc.dram_tensor("wv", [wv_rows, HID], MM_DT, kind="ExternalInput").ap()
    wo = nc.dram_tensor("wo", [HID, HID], MM_DT, kind="ExternalInput").ap()
    w1 = nc.dram_tensor("w1", [HID, H2], MM_DT, kind="ExternalInput").ap()
    w2 = nc.dram_tensor("w2", [H2, HID], MM_DT, kind="ExternalInput").ap()
    cst = nc.dram_tensor("cst", [128, _C_N], F32, kind="ExternalInput").ap()
    outT = nc.dram_tensor("outT", [HID, bl], MM_DT, kind="ExternalOutput").ap()
    mark = None
    if mark_out:
        mark = nc.dram_tensor("mark", [1, 8 * reps], F32,
                              kind="ExternalOutput").ap()

    n_kg = hs_rows // (KSUB * 128)
    hsT_t = hsT.rearrange("(kg kk p) b -> kg p kk b", kk=KSUB, p=128)
    hgT_t = hgT.rearrange("(c p) b -> p c b", p=128)
    outT_t = outT.rearrange("(c p) b -> p c b", p=128)

    with tile.TileContext(nc) as tc:
        with (
            nc.allow_low_precision(
                reason="bf16 matmuls/activations by design; fp32 accum"),
            tc.tile_pool(name="weights", bufs=1) as wpool,
            tc.tile_pool(name="hs", bufs=7) as hs_pool,
            tc.tile_pool(name="hg", bufs=2) as hg_pool,
            tc.tile_pool(name="v", bufs=6) as v_pool,
            tc.tile_pool(name="act", bufs=2) as act_pool,
            tc.tile_pool(name="res", bufs=2) as res_pool,
            tc.tile_pool(name="g", bufs=6) as g_pool,
            tc.tile_pool(name="xsq", bufs=3) as xsq_pool,
            tc.tile_pool(name="stat", bufs=2) as stat_pool,
            tc.tile_pool(name="out", bufs=2) as out_pool,
            tc.tile_pool(name="psum", bufs=8, space="PSUM") as psum,
        ):
            # ---- resident weights / constants ----
            # Weights ride the ACT HWDGE ring (nc.scalar) so they never
            # head-of-line-block h_s/h_g streaming on the SP ring; wv is
            # chunked so the first V matmuls start as soon as chunk 0 and
            # the first h_s tile land.
            # consts ride the SWDGE queue: tiny, needed by the V-phase
            # bias acts, and must not delay wv[0] on the ACT ring
            consts = wpool.tile([128, _C_N], F32)
            nc.gpsimd.dma_start(out=consts, in_=cst)
            wv_sb = wpool.tile([128, SK, HID], MM_DT)
            wv_r = wv.rearrange("(kc p) n -> p kc n", p=128)
            n_wv_kc = wv_rows // 128
            if n_wv_kc < SK:  # timing variant: replicate the small wv
                for j0 in range(0, SK, n_wv_kc):
                    nc.scalar.dma_start(out=wv_sb[:, j0:j0 + n_wv_kc, :],
                                        in_=wv_r)
            else:
                # first k-subtile rides alone so the first matmul can
                # start ~1us in; the rest follow in KSUB-sized chunks
                nc.scalar.dma_start(out=wv_sb[:, 0:1, :],
                                    in_=wv_r[:, 0:1, :])
                nc.scalar.dma_start(out=wv_sb[:, 1:KSUB, :],
                                    in_=wv_r[:, 1:KSUB, :])
                for j0 in range(KSUB, SK, KSUB):
                    nc.scalar.dma_start(out=wv_sb[:, j0:j0 + KSUB, :],
                                        in_=wv_r[:, j0:j0 + KSUB, :])
            wo_sb = wpool.tile([128, KO, HID], MM_DT)
            nc.scalar.dma_start(
                out=wo_sb, in_=wo.rearrange("(kc p) n -> p kc n", p=128))
            w1_sb = wpool.tile([128, KO, H2], MM_DT)
            nc.scalar.dma_start(
                out=w1_sb, in_=w1.rearrange("(kc p) n -> p kc n", p=128))
            w2_sb = wpool.tile([128, MH, HID], MM_DT)
            nc.scalar.dma_start(
                out=w2_sb, in_=w2.rearrange("(kc p) n -> p kc n", p=128))
            # (1/512)-scaled ones: stats matmuls produce means directly.
            # memset can't write bf16; produce via an ACT copy.
            ones_f = wpool.tile([128, 128], F32)
            nc.vector.memset(ones_f, 1.0 / (KO * 128))
            onesN = wpool.tile([128, 128], MM_DT)
            nc.scalar.activation(onesN, ones_f, AF.Copy)
            eps_col = wpool.tile([128, 1], F32)
            nc.vector.memset(eps_col, EPS)
            mark_sb = None
            if mark_out:
                mark_sb = wpool.tile([1, 8], F32)
                nc.vector.memset(mark_sb, 1.0)

            def _stats(x, tag):
                """Feature-axis mean/rstd of x [128, KO, NB] (bf16).
                Returns (mean, rstd) as bf16 SBUF tiles [128, NB] so the
                normalize ops run pure-bf16 at 2x DVE rate and the stats
                psums release early. x^2 rides the ACT engine (Square).
                Stats matmuls use (1/512)-scaled ones -> means directly."""
                muP = psum.tile([128, NB], F32, tag="psum_mm",
                                name=f"mu_{tag}")
                for j in range(KO):
                    nc.tensor.matmul(muP, onesN, x[:, j, :],
                                     start=(j == 0), stop=(j == KO - 1))
                sqP = psum.tile([128, NB], F32, tag="psum_mm",
                                name=f"sq_{tag}")
                for j in range(KO):
                    xsq = xsq_pool.tile([128, NB], MM_DT, tag="xsq",
                                        name=f"xsq_{tag}{j}")
                    nc.scalar.activation(xsq, x[:, j, :], AF.Square)
                    nc.tensor.matmul(sqP, onesN, xsq,
                                     start=(j == 0), stop=(j == KO - 1))
                mu_b = stat_pool.tile([128, NB], MM_DT, tag="mu",
                                      name=f"mu_{tag}")
                nc.scalar.activation(mu_b, muP, AF.Copy)
                varB = stat_pool.tile([128, NB], F32, tag="var",
                                      name=f"var_{tag}")
                # (walrus: an op may read PSUM at most once -> mu_b * muP)
                nc.vector.tensor_mul(varB, muP, mu_b)
                nc.vector.tensor_sub(varB, sqP, varB)
                sdv = stat_pool.tile([128, NB], MM_DT, tag="sdv",
                                     name=f"sdv_{tag}")
                nc.scalar.activation(sdv, varB, AF.Sqrt, bias=eps_col)
                rstd = stat_pool.tile([128, NB], MM_DT, tag="rstd",
                                      name=f"rstd_{tag}")
                nc.vector.reciprocal(rstd, sdv)
                return mu_b, rstd

            state = {}

            def finish_adds(st):
                """x2 = psum2 + x1r, emitted at the top of the next tile's
                iteration: highest DVE priority (DVE is idle during the V
                phase) and releases the psum_2 banks the V matmuls reuse.
                (PSUM is only readable by DVE/ACT - GPSIMD has no port.)"""
                rep, bt = st["id"]
                x2 = out_pool.tile([128, KO, NB], MM_DT, tag="x2",
                                   name=f"x2_{rep}_{bt}")
                st["x2"] = x2
                for m in range(KO):
                    nc.vector.tensor_add(x2[:, m, :], st["psum_2"][m],
                                         st["x1r"][:, m, :])

            def finish_a(st, last=False):
                """LN2 stats of the previous tile: the stats matmuls give
                the PE work while this tile's LN1 var chain runs."""
                rep, bt = st["id"]
                x2 = st["x2"]
                tag = f"ln2_{rep}_{bt}"
                muP = psum.tile([128, NB], F32, tag="psum_mm",
                                name=f"mu2_{tag}")
                sqP = psum.tile([128, NB], F32, tag="psum_mm",
                                name=f"sq2_{tag}")
                for m in range(KO):
                    nc.tensor.matmul(muP, onesN, x2[:, m, :],
                                     start=(m == 0), stop=(m == KO - 1))
                for m in range(KO):
                    xsq = xsq_pool.tile([128, NB], MM_DT, tag="xsq",
                                        name=f"xsq_{tag}{m}")
                    nc.scalar.activation(xsq, x2[:, m, :], AF.Square)
                    nc.tensor.matmul(sqP, onesN, xsq,
                                     start=(m == 0), stop=(m == KO - 1))
                st["muP"], st["sqP"] = muP, sqP

            def finish_b(st, last=False):
                """Part B: LN2 var/rstd, normalize+affine, store."""
                rep, bt = st["id"]
                bsl = st["bsl"]
                x2, muP, sqP = st["x2"], st["muP"], st["sqP"]
                tag = f"ln2_{rep}_{bt}"
                mu2 = stat_pool.tile([128, NB], MM_DT, tag="mu",
                                     name=f"mu_{tag}")
                nc.scalar.activation(mu2, muP, AF.Copy)
                varB = stat_pool.tile([128, NB], F32, tag="var",
                                      name=f"var_{tag}")
                nc.vector.tensor_mul(varB, muP, mu2)
                nc.vector.tensor_sub(varB, sqP, varB)
                sdv = stat_pool.tile([128, NB], MM_DT, tag="sdv",
                                     name=f"sdv_{tag}")
                nc.scalar.activation(sdv, varB, AF.Sqrt, bias=eps_col)
                rstd2 = stat_pool.tile([128, NB], MM_DT, tag="rstd",
                                       name=f"rstd_{tag}")
                nc.vector.reciprocal(rstd2, sdv)
                for m in range(KO):
                    eng = (nc.vector if m % 2 == 0 else nc.gpsimd) \
                        if last else nc.gpsimd
                    eng.tensor_sub(x2[:, m, :], x2[:, m, :], mu2)
                    eng.tensor_mul(x2[:, m, :], x2[:, m, :], rstd2)
                    # LN2 affine as one dual-scalar op (keeps ACT free)
                    eng.tensor_scalar(
                        x2[:, m, :], x2[:, m, :],
                        consts[:, _C_L2G + m: _C_L2G + m + 1],
                        consts[:, _C_L2B + m: _C_L2B + m + 1],
                        mybir.AluOpType.mult, mybir.AluOpType.add,
                    )
                    nc.scalar.dma_start(out=outT_t[:, m, bsl],
                                        in_=x2[:, m, :])

            for rep in range(reps):
              for bt in range(NBT):
                bsl = slice(0, NB) if timing else slice(bt * NB, (bt + 1) * NB)

                if state:
                    finish_adds(state)

                # ---- V = h_s @ Wv  (feature-major V^T in psum) ----
                psum_v = [psum.tile([128, NB], F32, tag="psum_mm",
                                    name=f"psv{rep}_{bt}_{i}")
                          for i in range(KO)]
                first = rep == 0 and bt == 0
                for kg in range(SK // KSUB):
                    hs_t = hs_pool.tile([128, KSUB, NB], MM_DT, name="hs_t")
                    if first and kg == 0:
                        # split the very first chunk so matmuls start early
                        nc.sync.dma_start(out=hs_t[:, 0:1, :],
                                          in_=hsT_t[0, :, 0:1, bsl])
                        nc.sync.dma_start(out=hs_t[:, 1:KSUB, :],
                                          in_=hsT_t[0, :, 1:KSUB, bsl])
                    else:
                        nc.sync.dma_start(out=hs_t,
                                          in_=hsT_t[kg % n_kg, :, :, bsl])
                    for kk in range(KSUB):
                        k = kg * KSUB + kk
                        for m in range(KO):
                            nc.tensor.matmul(
                                psum_v[m],
                                wv_sb[:, k, m * 128:(m + 1) * 128],
                                hs_t[:, kk, :],
                                start=(k == 0), stop=(k == SK - 1),
                            )

                v_sb = []
                for m in range(KO):
                    v = v_pool.tile([128, NB], MM_DT, tag="v",
                                    name=f"v{rep}_{bt}_{m}")
                    nc.scalar.activation(v, psum_v[m], AF.Identity,
                                         bias=consts[:, _C_BV + m: _C_BV + m + 1])
                    v_sb.append(v)

                # ---- x1 = V @ Wo + (h_g + bo) ----
                hg_t = hg_pool.tile([128, KO, NB], MM_DT, name="hg_t")
                nc.sync.dma_start(out=hg_t, in_=hgT_t[:, :, bsl])
                x1 = act_pool.tile([128, KO, NB], MM_DT, tag="x1", name="x1")
                for m in range(KO):
                    po = psum.tile([128, NB], F32, tag="psum_mm",
                                   name=f"pso{rep}_{bt}_{m}")
                    for k in range(KO):
                        nc.tensor.matmul(
                            po,
                            wo_sb[:, k, m * 128:(m + 1) * 128],
                            v_sb[k],
                            start=(k == 0), stop=(k == KO - 1),
                        )
                    nc.vector.tensor_add(x1[:, m, :], po, hg_t[:, m, :])

                # ---- LN1 stats (affine folded into W1/b1') ----
                mu1, rstd1 = _stats(x1, f"ln1_{rep}_{bt}")
                # previous tile's tail part A: its LN2 stats matmuls keep
                # the PE busy alongside this tile's stats
                if state:
                    finish_a(state)

                # ---- g = gelu(xhat @ W1' + b1') without waiting for LN:
                # since mu/rstd broadcast over the contraction (feature)
                # axis, xhat @ W1' == rstd * (x1 @ W1' - mu * colsum(W1')).
                # The W1 matmuls consume RAW x1 (no LN dependency) and the
                # normalization collapses into a per-chunk correction.
                g_sb = []
                for m in range(MH):
                    p1 = psum.tile([128, NB], F32, tag="psum_mm",
                                   name=f"ps1{rep}_{bt}_{m}")
                    for k in range(KO):
                        nc.tensor.matmul(
                            p1,
                            w1_sb[:, k, m * 128:(m + 1) * 128],
                            x1[:, k, :],
                            start=(k == 0), stop=(k == KO - 1),
                        )
                    t1 = xsq_pool.tile([128, NB], MM_DT, tag="corr",
                                       name=f"c{rep}_{bt}_{m}")
                    # t1 = (mu1 * -colsum(W1')[m]) + p1
                    nc.vector.scalar_tensor_tensor(
                        t1, mu1, consts[:, _C_W1S + m: _C_W1S + m + 1], p1,
                        mybir.AluOpType.mult, mybir.AluOpType.add)
                    nc.vector.tensor_mul(t1, t1, rstd1)
                    g = g_pool.tile([128, NB], MM_DT, tag="g",
                                    name=f"g{rep}_{bt}_{m}")
                    nc.scalar.activation(g, t1, AF.Gelu,
                                         bias=consts[:, _C_B1 + m: _C_B1 + m + 1])
                    g_sb.append(g)

                # residual carry (off-critical): x1r = xhat*g1 + (ln1_b+b2)
                x1r = res_pool.tile([128, KO, NB], MM_DT, tag="x1r",
                                    name=f"x1r_{rep}_{bt}")
                for m in range(KO):
                    nc.vector.tensor_sub(x1[:, m, :], x1[:, m, :], mu1)
                    nc.vector.tensor_mul(x1[:, m, :], x1[:, m, :], rstd1)
                    nc.gpsimd.tensor_scalar(
                        x1r[:, m, :], x1[:, m, :],
                        consts[:, _C_X1G + m: _C_X1G + m + 1],
                        consts[:, _C_X1B + m: _C_X1B + m + 1],
                        mybir.AluOpType.mult, mybir.AluOpType.add,
                    )

                # ---- x2 partial = g @ W2 (psum held into next V window) ----
                # k-outer hides the gelu pipeline behind the accumulation
                psum_2 = [psum.tile([128, NB], F32, tag="psum_mm",
                                    name=f"ps2{rep}_{bt}_{i}")
                          for i in range(KO)]
                for k in range(MH):
                    for m in range(KO):
                        nc.tensor.matmul(
                            psum_2[m],
                            w2_sb[:, k, m * 128:(m + 1) * 128],
                            g_sb[k],
                            start=(k == 0), stop=(k == MH - 1),
                        )

                # ---- previous tile's tail part B: emitted at tile end so
                # its side-engine work is lower priority than this tile's
                # critical chains.
                if state:
                    finish_b(state)
                state = {"psum_2": psum_2, "x1r": x1r, "bsl": bsl,
                         "id": (rep, bt)}

              if mark_out:
                nc.scalar.dma_start(out=mark[0:1, 8 * rep: 8 * (rep + 1)],
                                    in_=mark_sb)

            if state:
                finish_adds(state)
                finish_a(state, last=True)
                finish_b(state, last=True)

    if split_waits:
        _split_multi_waits(nc)
    return nc


def _chunk_cols(vec):
    """[n*128] -> [128, n] with column j = vec[j*128:(j+1)*128]."""
    return np.ascontiguousarray(vec.reshape(-1, 128).T.astype(np.float32))


NP_BF16 = mybir.dt.np(mybir.dt.bfloat16)


def _bf(x):
    return np.ascontiguousarray(np.asarray(x, np.float32).astype(NP_BF16))


def _make_consts(inputs):
    b1 = np.asarray(inputs["b1"], np.float32)
    b2 = np.asarray(inputs["b2"], np.float32)
    W1 = np.asarray(inputs["W1"], np.float32)
    ln1_g = np.asarray(inputs["ln1_g"], np.float32)
    ln1_b = np.asarray(inputs["ln1_b"], np.float32)
    bv_flat = np.asarray(inputs["bv"], np.float32).reshape(HID)
    b1_eff = b1 + ln1_b @ W1
    w1_eff = ln1_g[:, None] * W1
    cst = np.concatenate(
        [
            _chunk_cols(bv_flat),
            _chunk_cols(b1_eff),
            _chunk_cols(ln1_g),
            _chunk_cols(ln1_b + b2),
            _chunk_cols(np.asarray(inputs["ln2_g"], np.float32)),
            _chunk_cols(np.asarray(inputs["ln2_b"], np.float32)),
            _chunk_cols(-w1_eff.sum(axis=0)),
        ],
        axis=1,
    )
    assert cst.shape == (128, _C_N)
    return cst


def _shared_weights(inputs):
    Wv = np.asarray(inputs["Wv"], np.float32)
    W1 = np.asarray(inputs["W1"], np.float32)
    ln1_g = np.asarray(inputs["ln1_g"], np.float32)
    return {
        "wv": _bf(Wv.transpose(1, 0, 2).reshape(S_DIM, HID)),
        "wo": _bf(inputs["Wo"]),
        "w1": _bf(ln1_g[:, None] * W1),   # LN1 scale folded into W1
        "w2": _bf(inputs["W2"]),
        "cst": _make_consts(inputs),
    }


def _prepare_in_maps(inputs):
    h_g = np.asarray(inputs["h_g"], np.float32)
    h_s = np.asarray(inputs["h_s"], np.float32)
    bo = np.asarray(inputs["bo"], np.float32)
    shared = _shared_weights(inputs)
    in_maps = []
    for c in range(N_CORES):
        rows = slice(c * BL, (c + 1) * BL)
        in_maps.append({
            "hsT": _bf(h_s[rows].T),
            # fold bo into the h_g residual: x1 = V@Wo + (h_g + bo)
            "hgT": _bf(h_g[rows].T + bo[:, None]),
            **shared,
        })
    return in_maps


def _prepare_timing_in_maps(inputs):
    h_g = np.asarray(inputs["h_g"], np.float32)
    h_s = np.asarray(inputs["h_s"], np.float32)
    bo = np.asarray(inputs["bo"], np.float32)
    shared = _shared_weights(inputs)
    m = {
        "hsT": _bf(h_s[:NB, :KSUB * 128].T),
        "hgT": _bf(h_g[:NB].T + bo[:, None]),
        **shared,
    }
    m["wv"] = np.ascontiguousarray(m.pop("wv")[: S_DIM // 8])
    return [dict(m) for _ in range(N_CORES)]


def _assemble(results):
    return np.ascontiguousarray(
        np.concatenate([r["outT"].T for r in results], axis=0)
    ).astype(np.float32)


def run(inputs, trace=False):
    nc = build_nc()
    in_maps = _prepare_in_maps(inputs)
    res = run_bass_kernel_spmd(nc, in_maps, list(range(N_CORES)), trace=trace)
    return _assemble(res.results), res


def kernel(**inputs):
    out, _ = run(inputs, trace=False)
    return out
